# revision 40
# baseline (speedup 1.0000x reference)
"""DialogSeqAttnMatch Trainium2 kernel (8-core SPMD, L1-sharded).

Math (reference):
  dlg   = concat(xq, xa) reshaped (B*M, H); M = LQ+LA
  x_proj = relu(xd @ W.T + b);  y_proj = relu(dlg @ W.T + b)
  scores[b,l,k] = x_proj[b,l] . y_proj[k]  masked (causal: ts(k) >= b, padding)
                  + rw0*|b - ts(k)|  (row 0 zeroed)
  out = softmax_k(scores) @ dlg   (row 0 of alpha zeroed -> out[0] = 0)

Key simplifications used here:
  - In the causally valid region ts(k) < b, so |b-ts| = b - ts separates:
    exp(s + rw0*b - rw0*ts) = exp(s) * e^{rw0*b} * e^{-rw0*ts}.  The row
    factor cancels in softmax; the column factor phi_k is folded into the
    value rows on the host.
  - Padding mask: zero the (phi-scaled) value rows AND the appended
    ones-column on the host, so masked tokens drop out of numerator and
    denominator.
  - Causal mask: per (batch, k-chunk) memsets of the exp'd probability
    tile (token timesteps are 64-aligned so the cuts are at 64-row
    boundaries).
  - Output = (P @ [phi*v, phi]) -> divide columns 0:128 by column 128.

Device layout (per core, l-slice of 64 rows for all 32 batches):
  xdT   (128 d, 2048 (b,l))  f32r   scores computed TRANSPOSED: (k, l)
  dlgT  (128 d, 2048 k)      f32r
  x_projT = relu(Wt.T @ xdT + b): (128 h, 2048 (b,l)) f32r
  y_projT = relu(Wt.T @ dlgT + b): (128 h, 2048 k)    f32r
  groups of 4 batches (256 l-cols); k-chunks of 128; per chunk:
    scoresT psum (128k, 256l) = y_projT_chunk^T @ x_projT_group  [fp32r]
    pT = exp(scoresT - S0) bf16 (stacked 4 chunks per ACT instruction)
    out_psum(A/B) (128 l, 129) += pT_half^T @ dlg_aug_chunk      [bf16]
  normalize: out[:, :128] * recip(out[:, 128]) -> DMA out.
"""
import os
import sys

sys.path.insert(0, "/opt/trn_rl_repo")

import numpy as np
import ml_dtypes

import concourse.bass as bass
import concourse.tile as tile
import concourse.mybir as mybir
from concourse import bacc
from concourse.bass_utils import run_bass_kernel_spmd

F32 = mybir.dt.float32
F32R = mybir.dt.float32r
BF16 = mybir.dt.bfloat16

B, L1, LQ, LA, H = 32, 512, 32, 32, 128
M = LQ + LA              # 64 tokens per timestep
K = B * M                # 2048 flattened history
NCORES = 8
LC = L1 // NCORES        # 64 l-rows per core
S0 = 40.0                # exp shift (scores are >= 0, max ~50)
T0 = 16.0                # phi centering
NG = 8                   # batch groups of 4 (256 l-cols each)
STACK = 4                # k-chunks stacked per PSUM buffer / exp instruction

_NC_CACHE = None


def _chunks_of_group(g):
    return 2 * g + 2


DEBUG_TAPS = False


def _build():
    nc = bacc.Bacc("TRN2", target_bir_lowering=False, debug=False)

    # fused inputs (one DMA each, ordered by consumption):
    #   inp1 = [Wt (128) | bcol (1) | dlgT (2048)]  f32r-rounded bits
    #   inp2 = xdT (2048)                            f32r-rounded bits
    #   inp3 = dlga bf16 pairs packed in f32 slots (1032)
    inp0 = nc.dram_tensor("inp0", [H, 129], F32R, kind="ExternalInput").ap()
    inp1 = nc.dram_tensor("inp1", [H, K], F32R, kind="ExternalInput").ap()
    inp2 = nc.dram_tensor("inp2", [H, B * LC], F32R, kind="ExternalInput").ap()
    inp3 = nc.dram_tensor("inp3", [128, 16 * 129], BF16, kind="ExternalInput").ap()

    out = nc.dram_tensor("out", [B, LC, H], F32, kind="ExternalOutput").ap()
    out_flat = out.rearrange("b l d -> (b l) d")  # (2048, 128)
    if DEBUG_TAPS:
        dbg_proj = nc.dram_tensor("dbg_proj", [2, H, K], F32,
                                  kind="ExternalOutput").ap()
        dbg_ps = nc.dram_tensor("dbg_ps", [NG, 2, 128, 129], F32,
                                kind="ExternalOutput").ap()

    with tile.TileContext(nc) as tc:
        with tc.tile_pool(name="const", bufs=1) as cpool, \
             tc.tile_pool(name="pt", bufs=3) as ptpool, \
             tc.tile_pool(name="osb", bufs=3) as osbpool, \
             tc.tile_pool(name="ps_big", bufs=3, space="PSUM") as psb, \
             tc.tile_pool(name="ps_out", bufs=2, space="PSUM") as pso:

            negs0 = cpool.tile([128, 1], F32)
            nc.vector.memset(negs0[:], -S0)

            i0_sb = cpool.tile([H, 129], F32R)
            nc.sync.dma_start(i0_sb[:], inp0[:])
            wt_sb = i0_sb[:, 0:128]
            bcol_sb = i0_sb[:, 128:129].bitcast(F32)

            # piece-wise inputs + projections so the exp train starts early.
            PIECES = [(0, 512), (512, 1024), (1024, 1536), (1536, 2048)]
            dlgT_sb = cpool.tile([H, K], F32R)
            xdT_sb = cpool.tile([H, B * LC], F32R)
            i3_sb = cpool.tile([128, 16 * 129], BF16)
            dlga_sb = i3_sb[:]  # (128, 2064)

            # DMAs enqueued in consumption order (HWDGE drains FIFO)
            nc.sync.dma_start(dlgT_sb[:, 0:512], inp1[:, 0:512])
            nc.sync.dma_start(xdT_sb[:, 0:1024], inp2[:, 0:1024])
            nc.sync.dma_start(i3_sb[:, 0:1032], inp3[:, 0:1032])
            nc.sync.dma_start(dlgT_sb[:, 512:1024], inp1[:, 512:1024])
            nc.sync.dma_start(xdT_sb[:, 1024:2048], inp2[:, 1024:2048])
            nc.sync.dma_start(dlgT_sb[:, 1024:1536], inp1[:, 1024:1536])
            nc.sync.dma_start(dlgT_sb[:, 1536:2048], inp1[:, 1536:2048])
            nc.sync.dma_start(i3_sb[:, 1032:2064], inp3[:, 1032:2064])

            # projections: out[h, col] = relu(sum_d Wt[d, h] * inT[d, col] + b[h])
            yproj = cpool.tile([H, K], F32R)
            xproj = cpool.tile([H, B * LC], F32R)

            def emit_proj(dst, src, lo, hi, name):
                ps = psb.tile([128, STACK * 256], F32, tag="scps",
                              name=f"psproj_{name}{lo}")
                n = hi - lo
                nc.tensor.matmul(ps[:, 0:n], wt_sb, src[:, lo:hi],
                                 start=True, stop=True)
                if name == "y":
                    # ACT is idle during the prologue; DVE handles the x side
                    # concurrently.
                    nc.scalar.activation(dst[:, lo:hi], ps[:, 0:n],
                                         mybir.ActivationFunctionType.Relu,
                                         bias=bcol_sb, scale=1.0)
                else:
                    nc.vector.tensor_scalar(dst[:, lo:hi], ps[:, 0:n],
                                            bcol_sb, 0.0,
                                            op0=mybir.AluOpType.add,
                                            op1=mybir.AluOpType.max)

            next_piece = {"y": 0, "x": 0}

            def need_proj(name, upto):
                dst, src = ((yproj, dlgT_sb) if name == "y"
                            else (xproj, xdT_sb))
                while next_piece[name] < len(PIECES) and \
                        PIECES[next_piece[name]][0] < upto:
                    lo, hi = PIECES[next_piece[name]]
                    emit_proj(dst, src, lo, hi, name)
                    next_piece[name] += 1

            need_proj("y", 256)
            need_proj("x", 256)

            # flat stack list with lookahead-2 software pipeline:
            #   emit scores(i); process(i-2) = exp + causal memsets + out-MMs
            flat = []
            for g in range(NG):
                nchunks = _chunks_of_group(g)
                for s0 in range(0, nchunks, STACK):
                    flat.append((g, s0, min(STACK, nchunks - s0)))
            # split the very last stack so its out-matmuls overlap the final
            # exp instead of serializing after it
            g, s0, ns = flat[-1]
            if ns == 4:
                flat[-1] = (g, s0, 2)
                flat.append((g, s0 + 2, 2))

            state = {}   # g -> (psA, psB)
            tiles = {}   # i -> (ps, pt)

            def emit_scores(i):
                g, s0, ns = flat[i]
                xg = xproj[:, g * 256:(g + 1) * 256]
                ps = psb.tile([128, STACK * 256], F32, tag="scps")
                pt = ptpool.tile([128, STACK * 256], BF16, tag="pt")
                tiles[i] = (ps, pt)
                for k in range(ns):
                    c = s0 + k
                    nc.tensor.matmul(ps[:, k * 256:(k + 1) * 256],
                                     yproj[:, c * 128:(c + 1) * 128], xg,
                                     start=True, stop=True)

            def emit_process(i):
                g, s0, ns = flat[i]
                nchunks = _chunks_of_group(g)
                ps, pt = tiles.pop(i)
                nc.scalar.activation(pt[:, 0:ns * 256], ps[:, 0:ns * 256],
                                     mybir.ActivationFunctionType.Exp,
                                     bias=negs0[:], scale=1.0)
                for k in range(ns):
                    c = s0 + k
                    blk = pt[:, k * 256:(k + 1) * 256]
                    if c == nchunks - 2:
                        nc.gpsimd.memset(blk[:, 0:64], 0)
                        nc.gpsimd.memset(blk[64:128, 64:128], 0)
                    elif c == nchunks - 1:
                        # cols 0:128 are never read (psA matmul skipped)
                        nc.gpsimd.memset(blk[:, 128:192], 0)
                        nc.gpsimd.memset(blk[64:128, 192:256], 0)
                if s0 == 0:
                    # psA/psB share one PSUM bank: psA's start=True clears the
                    # bank's has_written bits, so psB's first matmul must use
                    # start=False (overwrites the still-clear region).
                    psAB = pso.tile([128, 260], F32, tag="psout", name=f"ps{g}")
                    state[g] = psAB
                psAB = state[g]
                psA = psAB[:, 0:129]
                psB = psAB[:, 130:259]
                for k in range(ns):
                    c = s0 + k
                    dchunk = dlga_sb[:, c * 129:(c + 1) * 129]
                    blk = pt[:, k * 256:(k + 1) * 256]
                    if c < nchunks - 1:
                        # last chunk's cols 0:128 are causally all-zero: skip
                        nc.tensor.matmul(psA, blk[:, 0:128], dchunk,
                                         start=(c == 0),
                                         stop=(c == nchunks - 2))
                    nc.tensor.matmul(psB, blk[:, 128:256], dchunk,
                                     start=False, stop=(c == nchunks - 1))
                if s0 + ns == nchunks:
                    emit_normalize(g)

            def emit_normalize(g):
                psAB = state.pop(g)
                psA = psAB[:, 0:129]
                psB = psAB[:, 130:259]
                osb = osbpool.tile([128, 256], F32, tag="osb")
                if DEBUG_TAPS:
                    for half, pshalf in ((0, psA), (1, psB)):
                        dcp = osbpool.tile([128, 129], F32, tag="dbgcp",
                                           name=f"dbgcp{g}_{half}")
                        nc.vector.tensor_copy(dcp[:], pshalf)
                        nc.sync.dma_start(dbg_ps[g, half], dcp[:])
                for half, pshalf in ((0, psA), (1, psB)):
                    dst = osb[:, half * 128:(half + 1) * 128]
                    recip = osbpool.tile([128, 1], F32, tag="recip")
                    if g == 0 and half == 0:
                        # batch 0: output is defined as zero (denominator is 0)
                        nc.vector.memset(dst[0:64, :], 0)
                        nc.vector.reciprocal(recip[64:128, :],
                                             pshalf[64:128, 128:129])
                        nc.vector.tensor_scalar_mul(dst[64:128, :],
                                                    pshalf[64:128, 0:128],
                                                    recip[64:128, :])
                    else:
                        nc.vector.reciprocal(recip[:], pshalf[:, 128:129])
                        nc.vector.tensor_scalar_mul(dst[:], pshalf[:, 0:128],
                                                    recip[:])
                dsl = out_flat[256 * g:256 * (g + 1)].rearrange(
                    "(h p) d -> p h d", h=2)
                nc.sync.dma_start(dsl, osb[:].rearrange("p (h d) -> p h d", h=2))

            LOOKAHEAD = 3
            for i in range(len(flat) + LOOKAHEAD):
                if i < len(flat):
                    g, s0, ns = flat[i]
                    need_proj("y", 128 * (s0 + ns))
                    need_proj("x", 256 * (g + 1))
                    emit_scores(i)
                j = i - LOOKAHEAD
                if 0 <= j < len(flat):
                    emit_process(j)

            if DEBUG_TAPS:
                nc.sync.dma_start(dbg_proj[0], xproj[:].bitcast(F32))
                nc.sync.dma_start(dbg_proj[1], yproj[:].bitcast(F32))

    nc.compile()
    return nc


def _get_nc():
    global _NC_CACHE
    if _NC_CACHE is None:
        _NC_CACHE = _build()
    return _NC_CACHE


def _round_f32r(a):
    u = np.ascontiguousarray(a, dtype=np.float32).view(np.uint32)
    r = ((u.astype(np.uint64) + 0x800) & 0xFFFFF000).astype(np.uint32)
    return r.view(np.float32)


LAST_RESULTS = None  # BassKernelResults of the most recent run (for test harness)


def kernel(xd_emb, xq_emb, xa_emb, W, b, recency_weight, xq_mask, xa_mask,
           _trace=False):
    xd_emb = np.asarray(xd_emb, np.float32)
    xq_emb = np.asarray(xq_emb, np.float32)
    xa_emb = np.asarray(xa_emb, np.float32)
    W = np.asarray(W, np.float32)
    b = np.asarray(b, np.float32)
    rw0 = float(np.asarray(recency_weight).reshape(-1)[0])
    pad = np.concatenate([np.asarray(xq_mask), np.asarray(xa_mask)], axis=1).reshape(K)

    dlg = np.concatenate([xq_emb, xa_emb], axis=1).reshape(K, H)
    ts = (np.arange(K) // M).astype(np.float64)
    phi = np.exp(-rw0 * (ts - T0))
    dlg_aug = np.concatenate([dlg.astype(np.float64), np.ones((K, 1))], axis=1)
    dlg_aug *= phi[:, None]
    dlg_aug[pad] = 0.0
    dlga_bf = dlg_aug.astype(ml_dtypes.bfloat16)
    dlga_packed = np.ascontiguousarray(
        dlga_bf.reshape(16, 128, 129).transpose(1, 0, 2).reshape(128, 16 * 129))

    inp0 = np.empty((H, 129), np.float32)
    inp0[:, 0:128] = _round_f32r(W.T)
    inp0[:, 128] = b
    inp1 = _round_f32r(dlg.T)
    inp3 = dlga_packed  # (128, 2064) bf16

    xdT = xd_emb.transpose(2, 0, 1)  # (H, B, L1)
    in_maps = []
    for c in range(NCORES):
        xdT_c = xdT[:, :, c * LC:(c + 1) * LC].reshape(H, B * LC)
        in_maps.append({
            "inp0": inp0,
            "inp1": inp1,
            "inp2": _round_f32r(xdT_c),
            "inp3": inp3,
        })

    nc = _get_nc()
    try:
        res = run_bass_kernel_spmd(nc, in_maps, list(range(NCORES)),
                                   trace=_trace)
    except ModuleNotFoundError:
        # The axon NTFF-profile hook is absent in this container; if an
        # ambient BASS_TRACE forced the trace path, retry without it.
        os.environ["BASS_NEVER_TRACE"] = "1"
        res = run_bass_kernel_spmd(nc, in_maps, list(range(NCORES)))
    global LAST_RESULTS
    LAST_RESULTS = res
    parts = [res.results[c]["out"] for c in range(NCORES)]
    full = np.concatenate(parts, axis=1)  # (32, 512, 128)
    full[0] = 0.0
    return np.ascontiguousarray(full, dtype=np.float32)


# revision 42
# speedup vs baseline: 1.0076x; 1.0076x over previous
"""DialogSeqAttnMatch Trainium2 kernel (8-core SPMD, L1-sharded).

Math (reference):
  dlg   = concat(xq, xa) reshaped (B*M, H); M = LQ+LA
  x_proj = relu(xd @ W.T + b);  y_proj = relu(dlg @ W.T + b)
  scores[b,l,k] = x_proj[b,l] . y_proj[k]  masked (causal: ts(k) >= b, padding)
                  + rw0*|b - ts(k)|  (row 0 zeroed)
  out = softmax_k(scores) @ dlg   (row 0 of alpha zeroed -> out[0] = 0)

Key simplifications used here:
  - In the causally valid region ts(k) < b, so |b-ts| = b - ts separates:
    exp(s + rw0*b - rw0*ts) = exp(s) * e^{rw0*b} * e^{-rw0*ts}.  The row
    factor cancels in softmax; the column factor phi_k is folded into the
    value rows on the host.
  - Padding mask: zero the (phi-scaled) value rows AND the appended
    ones-column on the host, so masked tokens drop out of numerator and
    denominator.
  - Causal mask: per (batch, k-chunk) memsets of the exp'd probability
    tile (token timesteps are 64-aligned so the cuts are at 64-row
    boundaries).
  - Output = (P @ [phi*v, phi]) -> divide columns 0:128 by column 128.

Device layout (per core, l-slice of 64 rows for all 32 batches):
  xdT   (128 d, 2048 (b,l))  f32r   scores computed TRANSPOSED: (k, l)
  dlgT  (128 d, 2048 k)      f32r
  x_projT = relu(Wt.T @ xdT + b): (128 h, 2048 (b,l)) f32r
  y_projT = relu(Wt.T @ dlgT + b): (128 h, 2048 k)    f32r
  groups of 4 batches (256 l-cols); k-chunks of 128; per chunk:
    scoresT psum (128k, 256l) = y_projT_chunk^T @ x_projT_group  [fp32r]
    pT = exp(scoresT - S0) bf16 (stacked 4 chunks per ACT instruction)
    out_psum(A/B) (128 l, 129) += pT_half^T @ dlg_aug_chunk      [bf16]
  normalize: out[:, :128] * recip(out[:, 128]) -> DMA out.
"""
import os
import sys

sys.path.insert(0, "/opt/trn_rl_repo")

import numpy as np
import ml_dtypes

import concourse.bass as bass
import concourse.tile as tile
import concourse.mybir as mybir
from concourse import bacc
from concourse.bass_utils import run_bass_kernel_spmd

F32 = mybir.dt.float32
F32R = mybir.dt.float32r
BF16 = mybir.dt.bfloat16

B, L1, LQ, LA, H = 32, 512, 32, 32, 128
M = LQ + LA              # 64 tokens per timestep
K = B * M                # 2048 flattened history
NCORES = 8
LC = L1 // NCORES        # 64 l-rows per core
S0 = 40.0                # exp shift (scores are >= 0, max ~50)
T0 = 16.0                # phi centering
NG = 8                   # batch groups of 4 (256 l-cols each)
STACK = 4                # k-chunks stacked per PSUM buffer / exp instruction

_NC_CACHE = None


def _chunks_of_group(g):
    return 2 * g + 2


DEBUG_TAPS = False


def _build():
    nc = bacc.Bacc("TRN2", target_bir_lowering=False, debug=False)

    # fused inputs (one DMA each, ordered by consumption):
    #   inp1 = [Wt (128) | bcol (1) | dlgT (2048)]  f32r-rounded bits
    #   inp2 = xdT (2048)                            f32r-rounded bits
    #   inp3 = dlga bf16 pairs packed in f32 slots (1032)
    inp0 = nc.dram_tensor("inp0", [H, 129], F32R, kind="ExternalInput").ap()
    inp1 = nc.dram_tensor("inp1", [H, K], F32R, kind="ExternalInput").ap()
    inp2 = nc.dram_tensor("inp2", [H, B * LC], F32R, kind="ExternalInput").ap()
    inp3 = nc.dram_tensor("inp3", [128, 16 * 129], BF16, kind="ExternalInput").ap()

    out = nc.dram_tensor("out", [B, LC, H], F32, kind="ExternalOutput").ap()
    out_flat = out.rearrange("b l d -> (b l) d")  # (2048, 128)
    if DEBUG_TAPS:
        dbg_proj = nc.dram_tensor("dbg_proj", [2, H, K], F32,
                                  kind="ExternalOutput").ap()
        dbg_ps = nc.dram_tensor("dbg_ps", [NG, 2, 128, 129], F32,
                                kind="ExternalOutput").ap()

    with tile.TileContext(nc) as tc:
        with tc.tile_pool(name="const", bufs=1) as cpool, \
             tc.tile_pool(name="pt", bufs=4) as ptpool, \
             tc.tile_pool(name="osb", bufs=4) as osbpool, \
             tc.tile_pool(name="ps_big", bufs=3, space="PSUM") as psb, \
             tc.tile_pool(name="ps_out", bufs=2, space="PSUM") as pso:

            negs0 = cpool.tile([128, 1], F32)
            nc.vector.memset(negs0[:], -S0)

            i0_sb = cpool.tile([H, 129], F32R)
            nc.sync.dma_start(i0_sb[:], inp0[:])
            wt_sb = i0_sb[:, 0:128]
            bcol_sb = i0_sb[:, 128:129].bitcast(F32)

            # piece-wise inputs + projections so the exp train starts early.
            PIECES = [(0, 512), (512, 1024), (1024, 1536), (1536, 2048)]
            dlgT_sb = cpool.tile([H, K], F32R)
            xdT_sb = cpool.tile([H, B * LC], F32R)
            i3_sb = cpool.tile([128, 16 * 129], BF16)
            dlga_sb = i3_sb[:]  # (128, 2064)

            # DMAs enqueued in consumption order (HWDGE drains FIFO)
            nc.sync.dma_start(dlgT_sb[:, 0:512], inp1[:, 0:512])
            nc.sync.dma_start(xdT_sb[:, 0:1024], inp2[:, 0:1024])
            nc.sync.dma_start(i3_sb[:, 0:1032], inp3[:, 0:1032])
            nc.sync.dma_start(dlgT_sb[:, 512:1024], inp1[:, 512:1024])
            nc.sync.dma_start(xdT_sb[:, 1024:2048], inp2[:, 1024:2048])
            nc.sync.dma_start(dlgT_sb[:, 1024:1536], inp1[:, 1024:1536])
            nc.sync.dma_start(dlgT_sb[:, 1536:2048], inp1[:, 1536:2048])
            nc.sync.dma_start(i3_sb[:, 1032:2064], inp3[:, 1032:2064])

            # projections: out[h, col] = relu(sum_d Wt[d, h] * inT[d, col] + b[h])
            yproj = cpool.tile([H, K], F32R)
            xproj = cpool.tile([H, B * LC], F32R)

            def emit_proj(dst, src, lo, hi, name):
                ps = psb.tile([128, STACK * 256], F32, tag="scps",
                              name=f"psproj_{name}{lo}")
                n = hi - lo
                nc.tensor.matmul(ps[:, 0:n], wt_sb, src[:, lo:hi],
                                 start=True, stop=True)
                if name == "y":
                    # ACT is idle during the prologue; DVE handles the x side
                    # concurrently.
                    nc.scalar.activation(dst[:, lo:hi], ps[:, 0:n],
                                         mybir.ActivationFunctionType.Relu,
                                         bias=bcol_sb, scale=1.0)
                else:
                    nc.vector.tensor_scalar(dst[:, lo:hi], ps[:, 0:n],
                                            bcol_sb, 0.0,
                                            op0=mybir.AluOpType.add,
                                            op1=mybir.AluOpType.max)

            next_piece = {"y": 0, "x": 0}

            def need_proj(name, upto):
                dst, src = ((yproj, dlgT_sb) if name == "y"
                            else (xproj, xdT_sb))
                while next_piece[name] < len(PIECES) and \
                        PIECES[next_piece[name]][0] < upto:
                    lo, hi = PIECES[next_piece[name]]
                    emit_proj(dst, src, lo, hi, name)
                    next_piece[name] += 1

            need_proj("y", 256)
            need_proj("x", 256)

            # flat stack list with lookahead-2 software pipeline:
            #   emit scores(i); process(i-2) = exp + causal memsets + out-MMs
            flat = []
            for g in range(NG):
                nchunks = _chunks_of_group(g)
                for s0 in range(0, nchunks, STACK):
                    flat.append((g, s0, min(STACK, nchunks - s0)))

            state = {}   # g -> (psA, psB)
            tiles = {}   # i -> (ps, pt)

            def emit_scores(i):
                g, s0, ns = flat[i]
                xg = xproj[:, g * 256:(g + 1) * 256]
                ps = psb.tile([128, STACK * 256], F32, tag="scps")
                pt = ptpool.tile([128, STACK * 256], BF16, tag="pt")
                tiles[i] = (ps, pt)
                for k in range(ns):
                    c = s0 + k
                    nc.tensor.matmul(ps[:, k * 256:(k + 1) * 256],
                                     yproj[:, c * 128:(c + 1) * 128], xg,
                                     start=True, stop=True)

            def emit_process(i):
                g, s0, ns = flat[i]
                nchunks = _chunks_of_group(g)
                ps, pt = tiles.pop(i)
                nc.scalar.activation(pt[:, 0:ns * 256], ps[:, 0:ns * 256],
                                     mybir.ActivationFunctionType.Exp,
                                     bias=negs0[:], scale=1.0)
                for k in range(ns):
                    c = s0 + k
                    blk = pt[:, k * 256:(k + 1) * 256]
                    if c == nchunks - 2:
                        nc.gpsimd.memset(blk[:, 0:64], 0)
                        nc.gpsimd.memset(blk[64:128, 64:128], 0)
                    elif c == nchunks - 1:
                        # cols 0:128 are never read (psA matmul skipped)
                        nc.gpsimd.memset(blk[:, 128:192], 0)
                        nc.gpsimd.memset(blk[64:128, 192:256], 0)
                if s0 == 0:
                    # psA/psB share one PSUM bank: psA's start=True clears the
                    # bank's has_written bits, so psB's first matmul must use
                    # start=False (overwrites the still-clear region).
                    psAB = pso.tile([128, 260], F32, tag="psout", name=f"ps{g}")
                    state[g] = psAB
                psAB = state[g]
                psA = psAB[:, 0:129]
                psB = psAB[:, 130:259]
                for k in range(ns):
                    c = s0 + k
                    dchunk = dlga_sb[:, c * 129:(c + 1) * 129]
                    blk = pt[:, k * 256:(k + 1) * 256]
                    if c < nchunks - 1:
                        # last chunk's cols 0:128 are causally all-zero: skip
                        nc.tensor.matmul(psA, blk[:, 0:128], dchunk,
                                         start=(c == 0),
                                         stop=(c == nchunks - 2))
                    nc.tensor.matmul(psB, blk[:, 128:256], dchunk,
                                     start=False, stop=(c == nchunks - 1))
                if s0 + ns == nchunks:
                    emit_normalize(g)

            def emit_normalize(g):
                psAB = state.pop(g)
                psA = psAB[:, 0:129]
                psB = psAB[:, 130:259]
                osb = osbpool.tile([128, 256], F32, tag="osb")
                if DEBUG_TAPS:
                    for half, pshalf in ((0, psA), (1, psB)):
                        dcp = osbpool.tile([128, 129], F32, tag="dbgcp",
                                           name=f"dbgcp{g}_{half}")
                        nc.vector.tensor_copy(dcp[:], pshalf)
                        nc.sync.dma_start(dbg_ps[g, half], dcp[:])
                for half, pshalf in ((0, psA), (1, psB)):
                    dst = osb[:, half * 128:(half + 1) * 128]
                    recip = osbpool.tile([128, 1], F32, tag="recip")
                    if g == 0 and half == 0:
                        # batch 0: output is defined as zero (denominator is 0)
                        nc.vector.memset(dst[0:64, :], 0)
                        nc.vector.reciprocal(recip[64:128, :],
                                             pshalf[64:128, 128:129])
                        nc.vector.tensor_scalar_mul(dst[64:128, :],
                                                    pshalf[64:128, 0:128],
                                                    recip[64:128, :])
                    else:
                        nc.vector.reciprocal(recip[:], pshalf[:, 128:129])
                        nc.vector.tensor_scalar_mul(dst[:], pshalf[:, 0:128],
                                                    recip[:])
                dsl = out_flat[256 * g:256 * (g + 1)].rearrange(
                    "(h p) d -> p h d", h=2)
                nc.sync.dma_start(dsl, osb[:].rearrange("p (h d) -> p h d", h=2))

            LOOKAHEAD = 3
            for i in range(len(flat) + LOOKAHEAD):
                if i < len(flat):
                    g, s0, ns = flat[i]
                    need_proj("y", 128 * (s0 + ns))
                    need_proj("x", 256 * (g + 1))
                    emit_scores(i)
                j = i - LOOKAHEAD
                if 0 <= j < len(flat):
                    emit_process(j)

            if DEBUG_TAPS:
                nc.sync.dma_start(dbg_proj[0], xproj[:].bitcast(F32))
                nc.sync.dma_start(dbg_proj[1], yproj[:].bitcast(F32))

    nc.compile()
    return nc


def _get_nc():
    global _NC_CACHE
    if _NC_CACHE is None:
        _NC_CACHE = _build()
    return _NC_CACHE


def _round_f32r(a):
    u = np.ascontiguousarray(a, dtype=np.float32).view(np.uint32)
    r = ((u.astype(np.uint64) + 0x800) & 0xFFFFF000).astype(np.uint32)
    return r.view(np.float32)


LAST_RESULTS = None  # BassKernelResults of the most recent run (for test harness)


def kernel(xd_emb, xq_emb, xa_emb, W, b, recency_weight, xq_mask, xa_mask,
           _trace=False):
    xd_emb = np.asarray(xd_emb, np.float32)
    xq_emb = np.asarray(xq_emb, np.float32)
    xa_emb = np.asarray(xa_emb, np.float32)
    W = np.asarray(W, np.float32)
    b = np.asarray(b, np.float32)
    rw0 = float(np.asarray(recency_weight).reshape(-1)[0])
    pad = np.concatenate([np.asarray(xq_mask), np.asarray(xa_mask)], axis=1).reshape(K)

    dlg = np.concatenate([xq_emb, xa_emb], axis=1).reshape(K, H)
    ts = (np.arange(K) // M).astype(np.float64)
    phi = np.exp(-rw0 * (ts - T0))
    dlg_aug = np.concatenate([dlg.astype(np.float64), np.ones((K, 1))], axis=1)
    dlg_aug *= phi[:, None]
    dlg_aug[pad] = 0.0
    dlga_bf = dlg_aug.astype(ml_dtypes.bfloat16)
    dlga_packed = np.ascontiguousarray(
        dlga_bf.reshape(16, 128, 129).transpose(1, 0, 2).reshape(128, 16 * 129))

    inp0 = np.empty((H, 129), np.float32)
    inp0[:, 0:128] = _round_f32r(W.T)
    inp0[:, 128] = b
    inp1 = _round_f32r(dlg.T)
    inp3 = dlga_packed  # (128, 2064) bf16

    xdT = xd_emb.transpose(2, 0, 1)  # (H, B, L1)
    in_maps = []
    for c in range(NCORES):
        xdT_c = xdT[:, :, c * LC:(c + 1) * LC].reshape(H, B * LC)
        in_maps.append({
            "inp0": inp0,
            "inp1": inp1,
            "inp2": _round_f32r(xdT_c),
            "inp3": inp3,
        })

    nc = _get_nc()
    try:
        res = run_bass_kernel_spmd(nc, in_maps, list(range(NCORES)),
                                   trace=_trace)
    except ModuleNotFoundError:
        # The axon NTFF-profile hook is absent in this container; if an
        # ambient BASS_TRACE forced the trace path, retry without it.
        os.environ["BASS_NEVER_TRACE"] = "1"
        res = run_bass_kernel_spmd(nc, in_maps, list(range(NCORES)))
    global LAST_RESULTS
    LAST_RESULTS = res
    parts = [res.results[c]["out"] for c in range(NCORES)]
    full = np.concatenate(parts, axis=1)  # (32, 512, 128)
    full[0] = 0.0
    return np.ascontiguousarray(full, dtype=np.float32)
